# revision 2
# baseline (speedup 1.0000x reference)
"""Trainium2 Bass kernel: GNN attention message-passing (AMP layer).

reference math (per node n, K neighbors):
    q      = nodes @ wq                       [N, FE]
    rq     = q @ wk.T = nodes @ (wq @ wk.T)   [N, FE]   (host folds wq@wk.T)
    logit[n,k] = inv_degree[n] * (edges[n,k,:] . rq[n,:])
    b      = softmax_k(logit)
    agg[n] = sum_k b[n,k] * nodes[nlist[n,k]]
    out    = agg @ wv
:
Distribution: node axis N sharded over 8 cores (6250 rows each, padded to
6272 = 49 tiles of 128). The full nodes table is replicated into every
core's DRAM; the neighbor gather is a per-core dma_gather. No collectives.

The gather uses int16 indices (hardware constraint), which cannot address
50000 rows directly, so the table is viewed as 25000 PAIR tokens of 2x128
floats and idx = nlist//2; the wrong half of each gathered pair is masked
out in the weighted-reduction coefficient matrix (even/odd split).

Perf structure (v2): the 16 DMA engines are the bottleneck (~7.5ns fixed +
bytes/20.5GBps per packet). All small per-tile streams (pair indices,
transposed self features, inv_degree, parity masks) are preloaded to SBUF
once in large packets; the output is accumulated in SBUF (partition-major,
bf16) and written once. Per tile only two DMAs remain: the edges stream
(4KB/partition) and the pair-token gather (2 x 2048-descriptor SWDGE
instructions on rotating queues). Pad slots index token 0 (always valid,
masked by zero coefficients) so every gather is full-size with no -1s.
"""

from contextlib import ExitStack

import ml_dtypes
import numpy as np

import concourse.bass as bass
import concourse.bacc as bacc
import concourse.tile as tile
from concourse import mybir
from concourse.bass_utils import run_bass_kernel_spmd

N, K, FN, FE = 50000, 32, 128, 64
NCORES = 8
SH = N // NCORES            # rows per core (6250)
TILE = 128
NT = -(-SH // TILE)         # tiles per core (49)
PAD = NT * TILE             # padded rows per core (6272)
CPG = TILE // K             # nodes completed per gather block (4)
NIDX = TILE * K             # gathered rows per tile (4096)
GPI = 2                     # gather instructions per tile
IPG = NIDX // GPI           # idxs per gather instruction (2048)
NWI = IPG // 16             # wrapped idx columns per instruction (128)

F32 = mybir.dt.float32
BF16 = mybir.dt.bfloat16
I16 = mybir.dt.int16

_CACHE: dict = {}


def _build_nc(n_table: int | None = None, nt: int | None = None):
    """Build the SPMD per-core graph. Identical on all 8 cores; only the
    DRAM input contents differ per core."""
    n_table = N if n_table is None else n_table
    nt = NT if nt is None else nt
    pad = nt * TILE
    npair = n_table // 2
    nc = bacc.Bacc(num_swdge_queues=4, dynamic_dma_scratch_size=32768)

    nodes_d = nc.dram_tensor("nodes", [npair, 2 * FN], BF16, kind="ExternalInput")
    xsT_d = nc.dram_tensor("xsTall", [FN, nt * TILE], BF16, kind="ExternalInput")
    edges_d = nc.dram_tensor("edges", [pad, K, FE], BF16, kind="ExternalInput")
    pidx_d = nc.dram_tensor("pidxall", [128, nt * GPI * NWI], I16, kind="ExternalInput")
    par_d = nc.dram_tensor("parall", [TILE, nt * K], BF16, kind="ExternalInput")
    inv_d = nc.dram_tensor("invall", [TILE, nt], F32, kind="ExternalInput")
    wv_d = nc.dram_tensor("wv", [FN, FN], BF16, kind="ExternalInput")
    wqkt_d = nc.dram_tensor("wqkt", [FN, FE], BF16, kind="ExternalInput")
    m128_d = nc.dram_tensor("m128", [TILE, TILE], BF16, kind="ExternalInput")
    i4t_d = nc.dram_tensor("i4t", [K, TILE], BF16, kind="ExternalInput")
    ident_d = nc.dram_tensor("ident", [TILE, TILE], F32, kind="ExternalInput")
    out_d = nc.dram_tensor("out", [TILE, nt * FN], BF16, kind="ExternalOutput")

    with tile.TileContext(nc) as tc, ExitStack() as ctx:
        consts = ctx.enter_context(tc.tile_pool(name="consts", bufs=1))
        big = ctx.enter_context(tc.tile_pool(name="big", bufs=3))
        gat = ctx.enter_context(tc.tile_pool(name="gat", bufs=3))
        med = ctx.enter_context(tc.tile_pool(name="med", bufs=3))
        small = ctx.enter_context(tc.tile_pool(name="small", bufs=4))
        psum = ctx.enter_context(tc.tile_pool(name="psum", bufs=1, space="PSUM"))

        wv_sb = consts.tile([FN, FN], BF16)
        nc.sync.dma_start(wv_sb[:], wv_d[:, :])
        wqkt_sb = consts.tile([FN, FE], BF16)
        nc.sync.dma_start(wqkt_sb[:], wqkt_d[:, :])
        m128_sb = consts.tile([TILE, TILE], BF16)
        nc.sync.dma_start(m128_sb[:], m128_d[:, :])
        i4t_sb = consts.tile([K, TILE], BF16)
        nc.sync.dma_start(i4t_sb[:], i4t_d[:, :])
        ident_sb = consts.tile([TILE, TILE], F32)
        nc.sync.dma_start(ident_sb[:], ident_d[:, :])

        # whole-run preloads (one big DMA each, large per-partition lines)
        pidx_sb = consts.tile([128, nt * GPI * NWI], I16)
        nc.sync.dma_start(pidx_sb[:], pidx_d[:, :])
        xsT_sb = consts.tile([FN, nt * TILE], BF16)
        nc.sync.dma_start(xsT_sb[:], xsT_d[:, :])
        par_sb = consts.tile([TILE, nt * K], BF16)
        nc.sync.dma_start(par_sb[:], par_d[:, :])
        iv_sb = consts.tile([TILE, nt], F32)
        nc.sync.dma_start(iv_sb[:], inv_d[:, :])

        # output accumulator, written to DRAM once at the end
        outacc = consts.tile([TILE, nt * FN], BF16)

        for t in range(nt):
            r0 = t * TILE

            # pair-token gather: stream element i of this tile lands at
            # xg[i%128, i//128, :]; all indices valid (pads point at token 0)
            xg = gat.tile([TILE, K, 2 * FN], BF16, tag="xg")
            for j in range(GPI):
                c0 = (t * GPI + j) * NWI
                nc.gpsimd.dma_gather(
                    xg[:, j * (K // GPI):(j + 1) * (K // GPI), :],
                    nodes_d[:, :],
                    pidx_sb[:, c0:c0 + NWI],
                    num_idxs=IPG, num_idxs_reg=IPG, elem_size=2 * FN,
                    single_packet=False, queue_num=(t * GPI + j) % 4,
                )

            # rq[n, c] = sum_f xself[n, f] * (wq@wk.T)[f, c]
            rq_ps = psum.tile([TILE, FE], F32, tag="rq_ps")
            nc.tensor.matmul(rq_ps[:], lhsT=xsT_sb[:, r0:r0 + TILE], rhs=wqkt_sb[:])
            rq = small.tile([TILE, FE], BF16, tag="rq")
            nc.scalar.copy(rq[:], rq_ps[:])

            # edges tile + logits: dots[n, k] = sum_c edges[n,k,c] * rq[n,c]
            ed = big.tile([TILE, K, FE], BF16, tag="ed")
            nc.sync.dma_start(ed[:], edges_d[r0:r0 + TILE, :, :])
            prod = big.tile([TILE, K, FE], BF16, tag="prod")
            rq_ap = rq[:]
            rq_bc = bass.AP(
                tensor=rq_ap.tensor,
                offset=rq_ap.offset,
                ap=[rq_ap.ap[0], [0, K], rq_ap.ap[1]],
            )
            nc.vector.tensor_tensor(
                out=prod[:], in0=ed[:], in1=rq_bc, op=mybir.AluOpType.mult
            )
            dots = small.tile([TILE, K], F32, tag="dots")
            nc.vector.tensor_reduce(
                out=dots[:], in_=prod[:], axis=mybir.AxisListType.X,
                op=mybir.AluOpType.add,
            )

            # scale by inv_degree, softmax over k (normalization deferred)
            scaled = small.tile([TILE, K], F32, tag="scaled")
            nc.scalar.mul(scaled[:], dots[:], iv_sb[:, t:t + 1])
            negmax = small.tile([TILE, 1], F32, tag="negmax")
            nc.vector.reduce_max(
                out=negmax[:], in_=scaled[:], axis=mybir.AxisListType.X, negate=True
            )
            expb = small.tile([TILE, K], F32, tag="expb")
            esum = small.tile([TILE, 1], F32, tag="esum")
            nc.scalar.activation(
                out=expb[:], in_=scaled[:], func=mybir.ActivationFunctionType.Exp,
                bias=negmax[:], scale=1.0, accum_out=esum[:],
            )
            rec = small.tile([TILE, 1], F32, tag="rec")
            nc.vector.reciprocal(rec[:], esum[:])

            # unnormalized coefficient matrix Bsel[r, j] = e[j, r%K] when
            # r//K == j%CPG else 0
            bT_ps = psum.tile([K, TILE], F32, tag="bT_ps")
            nc.tensor.transpose(bT_ps[:], expb[:], ident_sb[:])
            bT = small.tile([K, TILE], BF16, tag="bT")
            nc.scalar.copy(bT[:], bT_ps[:])
            brep_ps = psum.tile([TILE, TILE], F32, tag="brep_ps")
            nc.tensor.matmul(brep_ps[:], lhsT=i4t_sb[:], rhs=bT[:])
            bsel = med.tile([TILE, TILE], BF16, tag="bsel")
            nc.vector.tensor_tensor(
                out=bsel[:], in0=brep_ps[:], in1=m128_sb[:],
                op=mybir.AluOpType.mult,
            )

            # even/odd split by gathered-pair parity: par[r, g] applies to
            # Bsel columns j = 4g..4g+3
            parm_ap = par_sb[:, t * K:(t + 1) * K]
            par_bc = bass.AP(
                tensor=parm_ap.tensor,
                offset=parm_ap.offset,
                ap=[parm_ap.ap[0], parm_ap.ap[1], [0, CPG]],
            )
            bselo = med.tile([TILE, TILE], BF16, tag="bselo")
            nc.vector.tensor_tensor(
                out=bselo[:].rearrange("p (g c) -> p g c", c=CPG),
                in0=bsel[:].rearrange("p (g c) -> p g c", c=CPG),
                in1=par_bc,
                op=mybir.AluOpType.mult,
            )
            bsele = med.tile([TILE, TILE], BF16, tag="bsele")
            nc.vector.tensor_tensor(
                out=bsele[:], in0=bsel[:], in1=bselo[:],
                op=mybir.AluOpType.subtract,
            )

            # weighted neighbor reduction:
            # aggT[f, j] = sum_r xg[r, g(j), par*128 + f] * Bsel[r, j]
            aggT_ps = psum.tile([TILE, TILE], F32, tag="aggT_ps")
            for g in range(K):
                cols = slice(CPG * g, CPG * (g + 1))
                nc.tensor.matmul(
                    aggT_ps[:, cols], lhsT=xg[:, g, 0:FN], rhs=bsele[:, cols],
                    start=True, stop=False,
                )
                nc.tensor.matmul(
                    aggT_ps[:, cols], lhsT=xg[:, g, FN:2 * FN], rhs=bselo[:, cols],
                    start=False, stop=True,
                )
            aggT = med.tile([TILE, TILE], BF16, tag="aggT")
            nc.scalar.copy(aggT[:], aggT_ps[:])

            # final projection + softmax normalization into the accumulator:
            # out[n, fo] = (sum_f aggT[f, n] wv[f, fo]) / esum[n]
            out_ps = psum.tile([TILE, FN], F32, tag="out_ps")
            nc.tensor.matmul(out_ps[:], lhsT=aggT[:], rhs=wv_sb[:])
            nc.scalar.mul(outacc[:, t * FN:(t + 1) * FN], out_ps[:], rec[:])

        nc.sync.dma_start(out_d[:, :], outacc[:])

    nc.finalize()
    return nc


def _host_constants():
    r = np.arange(TILE)
    j = np.arange(TILE)
    m128 = (r[:, None] // K == j[None, :] % CPG).astype(ml_dtypes.bfloat16)
    i4t = (np.arange(TILE)[None, :] % K ==
           np.arange(K)[:, None]).astype(ml_dtypes.bfloat16)
    ident = np.eye(TILE, dtype=np.float32)
    return m128, i4t, ident


def _host_prep(inputs):
    nodes = np.ascontiguousarray(np.asarray(inputs["nodes"], dtype=np.float32))
    nlist = np.asarray(inputs["nlist"]).astype(np.int32)
    edges = np.asarray(inputs["edges"], dtype=np.float32)
    inv_degree = np.asarray(inputs["inv_degree"], dtype=np.float32)
    wq = np.asarray(inputs["wq"], dtype=np.float32)
    wk = np.asarray(inputs["wk"], dtype=np.float32)
    wv = np.asarray(inputs["wv"], dtype=np.float32)

    n_table = nodes.shape[0]
    wqkt = np.ascontiguousarray((wq @ wk.T).astype(np.float32))
    m128, i4t, ident = _host_constants()
    pair_view = np.ascontiguousarray(
        nodes.reshape(n_table // 2, 2 * FN).astype(ml_dtypes.bfloat16))

    in_maps = []
    for c in range(NCORES):
        lo = c * SH
        hi = lo + SH

        ed = np.zeros((PAD, K, FE), ml_dtypes.bfloat16)
        ed[:SH] = edges[lo:hi].astype(ml_dtypes.bfloat16)

        xs = np.zeros((PAD, FN), np.float32)
        xs[:SH] = nodes[lo:hi]
        # xsTall[f, t*128 + n] = xs[t*128 + n, f]
        xsTall = np.ascontiguousarray(
            xs.reshape(NT, TILE, FN).transpose(2, 0, 1)
            .reshape(FN, NT * TILE).astype(ml_dtypes.bfloat16))

        # ivall[p, t] = inv_degree[t*128 + p]
        iv = np.ones((PAD,), np.float32)
        iv[:SH] = inv_degree[lo:hi]
        ivall = np.ascontiguousarray(iv.reshape(NT, TILE).T)

        nl = np.zeros((PAD, K), np.int32)
        nl[:SH] = nlist[lo:hi]
        # per-tile gather stream: position i holds nlist[t*128 + i//K, i%K];
        # pad slots point at token 0 (valid row, zero coefficient)
        streams = nl.reshape(NT, NIDX)
        # wrapped int16 pair indices per gather instruction:
        # pidxall[16q + rep, (t*GPI + j)*NWI + s] = stream[t, j*IPG + s*16 + q]
        pidx16 = (streams // 2).astype(np.int16).reshape(
            NT * GPI, NWI, 16).transpose(0, 2, 1)     # [NT*GPI, 16, NWI]
        pidxall = np.ascontiguousarray(
            np.tile(pidx16, (1, 8, 1)).transpose(1, 0, 2)
            .reshape(128, NT * GPI * NWI))
        # parity parall[r, t*K + g] = stream[t, g*128+r] % 2
        parall = np.ascontiguousarray(
            (streams % 2).astype(ml_dtypes.bfloat16)
            .reshape(NT, K, TILE).transpose(2, 0, 1).reshape(TILE, NT * K)
        )

        in_maps.append({
            "nodes": pair_view,
            "xsTall": xsTall,
            "edges": ed,
            "pidxall": pidxall,
            "parall": parall,
            "invall": ivall,
            "wv": wv.astype(ml_dtypes.bfloat16),
            "wqkt": wqkt.astype(ml_dtypes.bfloat16),
            "m128": m128,
            "i4t": i4t,
            "ident": ident,
        })
    return in_maps


def _run(inputs, trace=False, **kw):
    nc = _CACHE.get("nc")
    if nc is None:
        nc = _build_nc()
        _CACHE["nc"] = nc
    in_maps = _host_prep(inputs)
    res = run_bass_kernel_spmd(
        nc, in_maps, core_ids=list(range(NCORES)), trace=trace, **kw
    )
    out = np.empty((N, FN), np.float32)
    for c in range(NCORES):
        # out DRAM is [128, NT*FN] bf16 partition-major; un-transpose on host
        o = np.asarray(res.results[c]["out"]).astype(np.float32)
        o = o.reshape(TILE, NT, FN).transpose(1, 0, 2).reshape(PAD, FN)
        out[c * SH:(c + 1) * SH] = o[:SH]
    return out, res


def kernel(**inputs) -> np.ndarray:
    out, _ = _run(inputs, trace=False)
    return out


# revision 3
# speedup vs baseline: 1.1257x; 1.1257x over previous
"""Trainium2 Bass kernel: GNN attention message-passing (AMP layer).

reference math (per node n, K neighbors):
    q      = nodes @ wq                       [N, FE]
    rq     = q @ wk.T = nodes @ (wq @ wk.T)   [N, FE]   (host folds wq@wk.T)
    logit[n,k] = inv_degree[n] * (edges[n,k,:] . rq[n,:])
    b      = softmax_k(logit)
    agg[n] = sum_k b[n,k] * nodes[nlist[n,k]]
    out    = agg @ wv
:
Distribution: node axis N sharded over 8 cores (6250 rows each, padded to
6272 = 49 tiles of 128). The full nodes table is replicated into every
core's DRAM; the neighbor gather is a per-core dma_gather. No collectives.

The gather uses int16 indices (hardware constraint), which cannot address
50000 rows directly, so the table is viewed as 25000 PAIR tokens of 2x128
floats and idx = nlist//2; the wrong half of each gathered pair is masked
out in the weighted-reduction coefficient matrix (even/odd split).

Perf structure (v2): the 16 DMA engines are the bottleneck (~7.5ns fixed +
bytes/20.5GBps per packet). All small per-tile streams (pair indices,
transposed self features, inv_degree, parity masks) are preloaded to SBUF
once in large packets; the output is accumulated in SBUF (partition-major,
bf16) and written once. Per tile only two DMAs remain: the edges stream
(4KB/partition) and the pair-token gather (2 x 2048-descriptor SWDGE
instructions on rotating queues). Pad slots index token 0 (always valid,
masked by zero coefficients) so every gather is full-size with no -1s.
"""

from contextlib import ExitStack

import ml_dtypes
import numpy as np

import concourse.bass as bass
import concourse.bacc as bacc
import concourse.tile as tile
from concourse import mybir
from concourse.bass_utils import run_bass_kernel_spmd

N, K, FN, FE = 50000, 32, 128, 64
NCORES = 8
SH = N // NCORES            # rows per core (6250)
TILE = 128
NT = -(-SH // TILE)         # tiles per core (49)
PAD = NT * TILE             # padded rows per core (6272)
CPG = TILE // K             # nodes completed per gather block (4)
NIDX = TILE * K             # gathered rows per tile (4096)
GPI = 4                     # gather instructions per tile
IPG = NIDX // GPI           # idxs per gather instruction (2048)
NWI = IPG // 16             # wrapped idx columns per instruction (128)

F32 = mybir.dt.float32
BF16 = mybir.dt.bfloat16
I16 = mybir.dt.int16

_CACHE: dict = {}


def _build_nc(n_table: int | None = None, nt: int | None = None):
    """Build the SPMD per-core graph. Identical on all 8 cores; only the
    DRAM input contents differ per core."""
    n_table = N if n_table is None else n_table
    nt = NT if nt is None else nt
    pad = nt * TILE
    npair = n_table // 2
    nc = bacc.Bacc(num_swdge_queues=4, dynamic_dma_scratch_size=32768)

    nodes_d = nc.dram_tensor("nodes", [npair, 2 * FN], BF16, kind="ExternalInput")
    xsT_d = nc.dram_tensor("xsTall", [FN, nt * TILE], BF16, kind="ExternalInput")
    edges_d = nc.dram_tensor("edges", [pad, K, FE], BF16, kind="ExternalInput")
    pidx_d = nc.dram_tensor("pidxall", [128, nt * GPI * NWI], I16, kind="ExternalInput")
    par_d = nc.dram_tensor("parall", [TILE, nt * K], BF16, kind="ExternalInput")
    inv_d = nc.dram_tensor("invall", [TILE, nt], F32, kind="ExternalInput")
    wv_d = nc.dram_tensor("wv", [FN, FN], BF16, kind="ExternalInput")
    wqkt_d = nc.dram_tensor("wqkt", [FN, FE], BF16, kind="ExternalInput")
    m128_d = nc.dram_tensor("m128", [TILE, TILE], BF16, kind="ExternalInput")
    i4t_d = nc.dram_tensor("i4t", [K, TILE], BF16, kind="ExternalInput")
    ident_d = nc.dram_tensor("ident", [TILE, TILE], F32, kind="ExternalInput")
    out_d = nc.dram_tensor("out", [TILE, nt * FN], BF16, kind="ExternalOutput")

    with tile.TileContext(nc) as tc, ExitStack() as ctx:
        consts = ctx.enter_context(tc.tile_pool(name="consts", bufs=1))
        big = ctx.enter_context(tc.tile_pool(name="big", bufs=3))
        gat = ctx.enter_context(tc.tile_pool(name="gat", bufs=3))
        med = ctx.enter_context(tc.tile_pool(name="med", bufs=3))
        small = ctx.enter_context(tc.tile_pool(name="small", bufs=4))
        psum = ctx.enter_context(tc.tile_pool(name="psum", bufs=1, space="PSUM"))

        wv_sb = consts.tile([FN, FN], BF16)
        nc.sync.dma_start(wv_sb[:], wv_d[:, :])
        wqkt_sb = consts.tile([FN, FE], BF16)
        nc.sync.dma_start(wqkt_sb[:], wqkt_d[:, :])
        m128_sb = consts.tile([TILE, TILE], BF16)
        nc.sync.dma_start(m128_sb[:], m128_d[:, :])
        i4t_sb = consts.tile([K, TILE], BF16)
        nc.sync.dma_start(i4t_sb[:], i4t_d[:, :])
        ident_sb = consts.tile([TILE, TILE], F32)
        nc.sync.dma_start(ident_sb[:], ident_d[:, :])

        # whole-run preloads (one big DMA each, large per-partition lines)
        pidx_sb = consts.tile([128, nt * GPI * NWI], I16)
        nc.sync.dma_start(pidx_sb[:], pidx_d[:, :])
        xsT_sb = consts.tile([FN, nt * TILE], BF16)
        nc.sync.dma_start(xsT_sb[:], xsT_d[:, :])
        par_sb = consts.tile([TILE, nt * K], BF16)
        nc.sync.dma_start(par_sb[:], par_d[:, :])
        iv_sb = consts.tile([TILE, nt], F32)
        nc.sync.dma_start(iv_sb[:], inv_d[:, :])

        # output accumulator, written to DRAM once at the end
        outacc = consts.tile([TILE, nt * FN], BF16)

        for t in range(nt):
            r0 = t * TILE

            # pair-token gather: stream element i of this tile lands at
            # xg[i%128, i//128, :]; all indices valid (pads point at token 0)
            xg = gat.tile([TILE, K, 2 * FN], BF16, tag="xg")
            for j in range(GPI):
                c0 = (t * GPI + j) * NWI
                nc.gpsimd.dma_gather(
                    xg[:, j * (K // GPI):(j + 1) * (K // GPI), :],
                    nodes_d[:, :],
                    pidx_sb[:, c0:c0 + NWI],
                    num_idxs=IPG, num_idxs_reg=IPG, elem_size=2 * FN,
                    single_packet=False, queue_num=(t * GPI + j) % 4,
                )

            # rq[n, c] = sum_f xself[n, f] * (wq@wk.T)[f, c]
            rq_ps = psum.tile([TILE, FE], F32, tag="rq_ps")
            nc.tensor.matmul(rq_ps[:], lhsT=xsT_sb[:, r0:r0 + TILE], rhs=wqkt_sb[:])
            rq = small.tile([TILE, FE], BF16, tag="rq")
            nc.scalar.copy(rq[:], rq_ps[:])

            # edges tile + logits: dots[n, k] = sum_c edges[n,k,c] * rq[n,c]
            ed = big.tile([TILE, K, FE], BF16, tag="ed")
            nc.sync.dma_start(ed[:], edges_d[r0:r0 + TILE, :, :])
            prod = big.tile([TILE, K, FE], BF16, tag="prod")
            rq_ap = rq[:]
            rq_bc = bass.AP(
                tensor=rq_ap.tensor,
                offset=rq_ap.offset,
                ap=[rq_ap.ap[0], [0, K], rq_ap.ap[1]],
            )
            nc.vector.tensor_tensor(
                out=prod[:], in0=ed[:], in1=rq_bc, op=mybir.AluOpType.mult
            )
            dots = small.tile([TILE, K], F32, tag="dots")
            nc.vector.tensor_reduce(
                out=dots[:], in_=prod[:], axis=mybir.AxisListType.X,
                op=mybir.AluOpType.add,
            )

            # scale by inv_degree, softmax over k (normalization deferred)
            scaled = small.tile([TILE, K], F32, tag="scaled")
            nc.scalar.mul(scaled[:], dots[:], iv_sb[:, t:t + 1])
            negmax = small.tile([TILE, 1], F32, tag="negmax")
            nc.vector.reduce_max(
                out=negmax[:], in_=scaled[:], axis=mybir.AxisListType.X, negate=True
            )
            expb = small.tile([TILE, K], F32, tag="expb")
            esum = small.tile([TILE, 1], F32, tag="esum")
            nc.scalar.activation(
                out=expb[:], in_=scaled[:], func=mybir.ActivationFunctionType.Exp,
                bias=negmax[:], scale=1.0, accum_out=esum[:],
            )
            rec = small.tile([TILE, 1], F32, tag="rec")
            nc.vector.reciprocal(rec[:], esum[:])

            # unnormalized coefficient matrix Bsel[r, j] = e[j, r%K] when
            # r//K == j%CPG else 0
            bT_ps = psum.tile([K, TILE], F32, tag="bT_ps")
            nc.tensor.transpose(bT_ps[:], expb[:], ident_sb[:])
            bT = small.tile([K, TILE], BF16, tag="bT")
            nc.scalar.copy(bT[:], bT_ps[:])
            brep_ps = psum.tile([TILE, TILE], F32, tag="brep_ps")
            nc.tensor.matmul(brep_ps[:], lhsT=i4t_sb[:], rhs=bT[:])
            bsel = med.tile([TILE, TILE], BF16, tag="bsel")
            nc.vector.tensor_tensor(
                out=bsel[:], in0=brep_ps[:], in1=m128_sb[:],
                op=mybir.AluOpType.mult,
            )

            # even/odd split by gathered-pair parity: par[r, g] applies to
            # Bsel columns j = 4g..4g+3
            parm_ap = par_sb[:, t * K:(t + 1) * K]
            par_bc = bass.AP(
                tensor=parm_ap.tensor,
                offset=parm_ap.offset,
                ap=[parm_ap.ap[0], parm_ap.ap[1], [0, CPG]],
            )
            bselo = med.tile([TILE, TILE], BF16, tag="bselo")
            nc.vector.tensor_tensor(
                out=bselo[:].rearrange("p (g c) -> p g c", c=CPG),
                in0=bsel[:].rearrange("p (g c) -> p g c", c=CPG),
                in1=par_bc,
                op=mybir.AluOpType.mult,
            )
            bsele = med.tile([TILE, TILE], BF16, tag="bsele")
            nc.vector.tensor_tensor(
                out=bsele[:], in0=bsel[:], in1=bselo[:],
                op=mybir.AluOpType.subtract,
            )

            # weighted neighbor reduction:
            # aggT[f, j] = sum_r xg[r, g(j), par*128 + f] * Bsel[r, j]
            aggT_ps = psum.tile([TILE, TILE], F32, tag="aggT_ps")
            for g in range(K):
                cols = slice(CPG * g, CPG * (g + 1))
                nc.tensor.matmul(
                    aggT_ps[:, cols], lhsT=xg[:, g, 0:FN], rhs=bsele[:, cols],
                    start=True, stop=False,
                )
                nc.tensor.matmul(
                    aggT_ps[:, cols], lhsT=xg[:, g, FN:2 * FN], rhs=bselo[:, cols],
                    start=False, stop=True,
                )
            aggT = med.tile([TILE, TILE], BF16, tag="aggT")
            nc.scalar.copy(aggT[:], aggT_ps[:])

            # final projection + softmax normalization into the accumulator:
            # out[n, fo] = (sum_f aggT[f, n] wv[f, fo]) / esum[n]
            out_ps = psum.tile([TILE, FN], F32, tag="out_ps")
            nc.tensor.matmul(out_ps[:], lhsT=aggT[:], rhs=wv_sb[:])
            nc.scalar.mul(outacc[:, t * FN:(t + 1) * FN], out_ps[:], rec[:])

        nc.sync.dma_start(out_d[:, :], outacc[:])

    nc.finalize()
    return nc


def _host_constants():
    r = np.arange(TILE)
    j = np.arange(TILE)
    m128 = (r[:, None] // K == j[None, :] % CPG).astype(ml_dtypes.bfloat16)
    i4t = (np.arange(TILE)[None, :] % K ==
           np.arange(K)[:, None]).astype(ml_dtypes.bfloat16)
    ident = np.eye(TILE, dtype=np.float32)
    return m128, i4t, ident


def _host_prep(inputs):
    nodes = np.ascontiguousarray(np.asarray(inputs["nodes"], dtype=np.float32))
    nlist = np.asarray(inputs["nlist"]).astype(np.int32)
    edges = np.asarray(inputs["edges"], dtype=np.float32)
    inv_degree = np.asarray(inputs["inv_degree"], dtype=np.float32)
    wq = np.asarray(inputs["wq"], dtype=np.float32)
    wk = np.asarray(inputs["wk"], dtype=np.float32)
    wv = np.asarray(inputs["wv"], dtype=np.float32)

    n_table = nodes.shape[0]
    wqkt = np.ascontiguousarray((wq @ wk.T).astype(np.float32))
    m128, i4t, ident = _host_constants()
    pair_view = np.ascontiguousarray(
        nodes.reshape(n_table // 2, 2 * FN).astype(ml_dtypes.bfloat16))

    in_maps = []
    for c in range(NCORES):
        lo = c * SH
        hi = lo + SH

        ed = np.zeros((PAD, K, FE), ml_dtypes.bfloat16)
        ed[:SH] = edges[lo:hi].astype(ml_dtypes.bfloat16)

        xs = np.zeros((PAD, FN), np.float32)
        xs[:SH] = nodes[lo:hi]
        # xsTall[f, t*128 + n] = xs[t*128 + n, f]
        xsTall = np.ascontiguousarray(
            xs.reshape(NT, TILE, FN).transpose(2, 0, 1)
            .reshape(FN, NT * TILE).astype(ml_dtypes.bfloat16))

        # ivall[p, t] = inv_degree[t*128 + p]
        iv = np.ones((PAD,), np.float32)
        iv[:SH] = inv_degree[lo:hi]
        ivall = np.ascontiguousarray(iv.reshape(NT, TILE).T)

        nl = np.zeros((PAD, K), np.int32)
        nl[:SH] = nlist[lo:hi]
        # per-tile gather stream: position i holds nlist[t*128 + i//K, i%K];
        # pad slots point at token 0 (valid row, zero coefficient)
        streams = nl.reshape(NT, NIDX)
        # wrapped int16 pair indices per gather instruction:
        # pidxall[16q + rep, (t*GPI + j)*NWI + s] = stream[t, j*IPG + s*16 + q]
        pidx16 = (streams // 2).astype(np.int16).reshape(
            NT * GPI, NWI, 16).transpose(0, 2, 1)     # [NT*GPI, 16, NWI]
        pidxall = np.ascontiguousarray(
            np.tile(pidx16, (1, 8, 1)).transpose(1, 0, 2)
            .reshape(128, NT * GPI * NWI))
        # parity parall[r, t*K + g] = stream[t, g*128+r] % 2
        parall = np.ascontiguousarray(
            (streams % 2).astype(ml_dtypes.bfloat16)
            .reshape(NT, K, TILE).transpose(2, 0, 1).reshape(TILE, NT * K)
        )

        in_maps.append({
            "nodes": pair_view,
            "xsTall": xsTall,
            "edges": ed,
            "pidxall": pidxall,
            "parall": parall,
            "invall": ivall,
            "wv": wv.astype(ml_dtypes.bfloat16),
            "wqkt": wqkt.astype(ml_dtypes.bfloat16),
            "m128": m128,
            "i4t": i4t,
            "ident": ident,
        })
    return in_maps


def _run(inputs, trace=False, **kw):
    nc = _CACHE.get("nc")
    if nc is None:
        nc = _build_nc()
        _CACHE["nc"] = nc
    in_maps = _host_prep(inputs)
    res = run_bass_kernel_spmd(
        nc, in_maps, core_ids=list(range(NCORES)), trace=trace, **kw
    )
    out = np.empty((N, FN), np.float32)
    for c in range(NCORES):
        # out DRAM is [128, NT*FN] bf16 partition-major; un-transpose on host
        o = np.asarray(res.results[c]["out"]).astype(np.float32)
        o = o.reshape(TILE, NT, FN).transpose(1, 0, 2).reshape(PAD, FN)
        out[c * SH:(c + 1) * SH] = o[:SH]
    return out, res


def kernel(**inputs) -> np.ndarray:
    out, _ = _run(inputs, trace=False)
    return out
